# revision 7
# baseline (speedup 1.0000x reference)
"""3-layer GCN (GCNConv x3) on Trainium2, distributed across 8 NeuronCores.

Strategy (graph/data parallel, per the sharding hint):
  - Nodes are block-partitioned across the 8 cores; each core owns the
    destination side (scatter-add aggregation) for its node shard.
  - The tiny weight matrices are replicated.
  - Per layer, each core computes g = dis * h for its shard, the shards are
    AllGathered into a shared HBM table G, and each core then aggregates
    messages for its own nodes with dma_gather (256B rows from G) +
    dma_scatter_add (CCE-add into its HBM accumulator).
  - Symmetric normalization is folded per-node:
       (Ahat h)[c] = dis[c] * ( sum_{e->c} dis[r] h[r] + dis[c] h[c] )
    so no per-edge multiplies are needed on-device.
  - Scatter-add rounds are built so destinations are distinct within each
    call (race-free CCE accumulation); high-degree columns spill into a
    second "virtual" level that is summed during readback.
"""
import sys
import numpy as np

sys.path.insert(0, "/opt/trn_rl_repo")

F = 64           # feature width (STATE == HID == 64)
NCORES = 8


class Plan:
    """Static (compile-time) layout shared by all cores, plus per-core data."""

    def __init__(self, n_nodes, edge_index, cap_min=24, row_chunk=32768):
        self.n_nodes = n_nodes
        self.row_chunk = row_chunk
        shard = (n_nodes + NCORES - 1) // NCORES          # real nodes per core
        sh = ((shard + 127) // 128) * 128                 # padded shard, x128
        self.shard, self.sh, self.t = shard, sh, sh // 128
        self.gtbl_rows = ((sh * NCORES + row_chunk - 1) // row_chunk) * row_chunk
        self.n_chunks = self.gtbl_rows // row_chunk

        row = np.asarray(edge_index[0], dtype=np.int64)
        col = np.asarray(edge_index[1], dtype=np.int64)
        deg = np.bincount(col, minlength=n_nodes).astype(np.float64) + 1.0
        self.dis = (1.0 / np.sqrt(deg)).astype(np.float32)

        tpos_row = (row // shard) * sh + (row % shard)    # global table row of source
        dst_core = col // shard
        cloc = col % shard

        # per-core: occurrence index of each edge within its destination column
        per_core = []
        maxdeg = 0
        for k in range(NCORES):
            m = dst_core == k
            r_k, c_k = tpos_row[m], cloc[m]
            o = np.argsort(c_k, kind="stable")
            cs = c_k[o]
            if cs.size:
                starts = np.r_[True, cs[1:] != cs[:-1]]
                run_starts = np.flatnonzero(starts)
                rid = np.cumsum(starts) - 1
                occ_s = np.arange(cs.size) - run_starts[rid]
                occ = np.empty_like(occ_s)
                occ[o] = occ_s
                maxdeg = max(maxdeg, int(occ_s.max()) + 1)
            else:
                occ = np.zeros(0, np.int64)
            per_core.append((r_k, c_k, occ))

        self.levels = 2
        self.cap = max(cap_min, (maxdeg + self.levels - 1) // self.levels)
        assert maxdeg <= self.cap * self.levels
        self.trash = self.levels * sh
        self.agg_rows = self.trash + 128
        assert self.agg_rows - 1 <= 32767

        # static per-(round, chunk) sizes = max over cores, padded to 128
        cnt = np.zeros((NCORES, self.cap, self.n_chunks), np.int64)
        for k, (r_k, c_k, occ) in enumerate(per_core):
            rnd = occ % self.cap
            chk = r_k // self.row_chunk
            np.add.at(cnt[k], (rnd, chk), 1)
        n_rc = cnt.max(axis=0)
        n_rc = np.maximum((n_rc + 127) // 128 * 128, 128)  # >=128, x128
        # pad each round's total to x512 so the scatter splits into 4 x128 sub-calls
        tot_r = n_rc.sum(axis=1)
        bump = (-tot_r) % 512
        n_rc[:, -1] += bump
        self.n_rc = n_rc                                   # [cap, n_chunks]
        self.n_r = n_rc.sum(axis=1)                        # tokens per round
        self.max_nr = int(self.n_r.max())
        self.tot_tok = int(self.n_r.sum())

        # token slot offsets
        self.base_r = np.concatenate([[0], np.cumsum(self.n_r)])[:-1]
        self.off_rc = np.concatenate(
            [np.zeros((self.cap, 1), np.int64), np.cumsum(n_rc, axis=1)], axis=1
        )[:, :-1]

        # per-core index arrays in the wrapped [128, n/16] int16 layout
        row_chunk_l = row_chunk
        self.gidx = []
        self.sidx = []
        for k, (r_k, c_k, occ) in enumerate(per_core):
            rnd = (occ % self.cap).astype(np.int64)
            lev = (occ // self.cap).astype(np.int64)
            chk = (r_k // self.row_chunk).astype(np.int64)
            gflat = np.zeros(self.tot_tok, np.int64)           # pad -> row 0 of chunk
            sflat = np.full(self.tot_tok, self.trash, np.int64)  # pad -> trash row
            # position of each edge: stable order within (round, chunk) buckets
            key = rnd * self.n_chunks + chk
            order = np.argsort(key, kind="stable")
            ks = key[order]
            if ks.size:
                starts = np.r_[True, ks[1:] != ks[:-1]]
                run_starts = np.flatnonzero(starts)
                rid = np.cumsum(starts) - 1
                within = np.arange(ks.size) - run_starts[rid]
                rr, cc = ks // self.n_chunks, ks % self.n_chunks
                pos = self.base_r[rr] + self.off_rc[rr, cc] + within
                e = order  # original edge ids in sorted order
                gflat[pos] = r_k[e] - cc * row_chunk_l
                sflat[pos] = lev[e] * sh + c_k[e]
            self.gidx.append(self._wrap(gflat))
            self.sidx.append(self._wrap(sflat))

        # slot offsets (in the [128, X] wrapped tensors) for each call
        self.gso = {}
        self.sso = {}
        off = 0
        for r in range(self.cap):
            for c in range(self.n_chunks):
                self.gso[(r, c)] = off
                off += int(n_rc[r, c]) // 16
        self.gslots = off
        off = 0
        for r in range(self.cap):
            self.sso[r] = off
            off += int(self.n_r[r]) // 16
        self.sslots = off

    @staticmethod
    def _wrap(idx):
        n = idx.size
        a = idx.astype(np.int16).reshape(n // 16, 16).T
        return np.ascontiguousarray(np.tile(a, (8, 1)))   # [128, n/16]

    def core_inputs(self, k, x, W1, b1, W2, b2, W3, b3):
        sh, shard, t = self.sh, self.shard, self.t
        xs = np.zeros((sh, F), np.float32)
        lo, hi = k * shard, min((k + 1) * shard, self.n_nodes)
        xs[: hi - lo] = x[lo:hi]
        ds = np.zeros(sh, np.float32)
        ds[: hi - lo] = self.dis[lo:hi]
        return {
            "x": np.ascontiguousarray(xs.reshape(128, t * F)),
            "dis": np.ascontiguousarray(ds.reshape(128, t)),
            "gidx": self.gidx[k],
            "sidx": self.sidx[k],
            "W1": np.asarray(W1, np.float32),
            "b1": np.asarray(b1, np.float32).reshape(F, 1),
            "W2": np.asarray(W2, np.float32),
            "b2": np.asarray(b2, np.float32).reshape(F, 1),
            "W3": np.asarray(W3, np.float32).reshape(F, 1),
            "b3": np.asarray(b3, np.float32).reshape(1, 1),
        }

    def assemble(self, outs):
        """outs: list per core of {'out': [sh]} in fm order (col 128*t+p = node p*T+t)."""
        sh, t = self.sh, self.t
        res = np.zeros((self.n_nodes, 1), np.float32)
        for k in range(NCORES):
            o = np.asarray(outs[k]["out"]).reshape(t, 128)  # [t, p]
            node_major = o.T.reshape(sh)                    # node l = p*t + t_idx
            lo = k * self.shard
            hi = min(lo + self.shard, self.n_nodes)
            res[lo:hi, 0] = node_major[: hi - lo]
        return res


def build(plan, n_layers=3):
    import concourse.bass as bass
    import concourse.bacc as bacc
    import concourse.mybir as mybir
    import concourse.tile as tile
    from concourse.masks import make_identity

    f32 = mybir.dt.float32
    i16 = mybir.dt.int16
    T, SH = plan.t, plan.sh
    CAP, NCH = plan.cap, plan.n_chunks
    MAXNR = plan.max_nr

    nc = bacc.Bacc("TRN2", target_bir_lowering=False, debug=False,
                   num_devices=NCORES, num_swdge_queues=2)

    x_t = nc.dram_tensor("x", [128, T * F], f32, kind="ExternalInput")
    dis_t = nc.dram_tensor("dis", [128, T], f32, kind="ExternalInput")
    gidx_t = nc.dram_tensor("gidx", [128, plan.gslots], i16, kind="ExternalInput")
    sidx_t = nc.dram_tensor("sidx", [128, plan.sslots], i16, kind="ExternalInput")
    Ws = {}
    for nm, shape in [("W1", [F, F]), ("b1", [F, 1]), ("W2", [F, F]),
                      ("b2", [F, 1]), ("W3", [F, 1]), ("b3", [1, 1])]:
        Ws[nm] = nc.dram_tensor(nm, shape, f32, kind="ExternalInput")
    out_t = nc.dram_tensor("out", [SH], f32, kind="ExternalOutput")

    g_dram = nc.dram_tensor("g_bounce", [SH * F], f32, kind="Internal")
    G = nc.dram_tensor("G_table", [plan.gtbl_rows, F], f32, kind="Internal",
                       addr_space="Shared")
    # two agg sets (layer parity) x four parity buffers (scatter sub-call index)
    NSUB = 4
    aggs = [[nc.dram_tensor(f"agg_{s}_{p}", [plan.agg_rows, F], f32,
                            kind="Internal") for p in range(NSUB)] for s in range(2)]

    rg = [list(range(NCORES))]

    with tile.TileContext(nc) as tc:
        with tc.tile_pool(name="const", bufs=1) as cpool, \
             tc.tile_pool(name="state", bufs=1) as spool, \
             tc.tile_pool(name="msg", bufs=2) as mpool, \
             tc.tile_pool(name="idx", bufs=2) as ipool, \
             tc.tile_pool(name="fm", bufs=2) as fpool, \
             tc.tile_pool(name="psum", bufs=2, space="PSUM") as ppool:

            ident = cpool.tile([128, 128], f32)
            make_identity(nc, ident[:])
            dis_s = cpool.tile([128, T], f32)
            nc.sync.dma_start(dis_s[:], dis_t[:])
            wsb = {}
            for nm in ("W1", "W2", "W3", "b1", "b2", "b3"):
                wsb[nm] = cpool.tile(list(Ws[nm].shape), f32, name=f"sb_{nm}")
                nc.sync.dma_start(wsb[nm][:], Ws[nm][:])

            dis_b = dis_s[:].unsqueeze(-1).broadcast_to([128, T, F])

            # g0 = x * dis
            g = spool.tile([128, T, F], f32, tag="g")
            tmp = spool.tile([128, T, F], f32, tag="tmp")
            nc.sync.dma_start(tmp[:], x_t[:].rearrange("p (t f) -> p t f", f=F))
            nc.vector.tensor_tensor(g[:], tmp[:], dis_b, mybir.AluOpType.mult)

            def zero_agg_set(s):
                z = mpool.tile([128, MAXNR // 128, F], f32, tag="msg")
                nc.vector.memset(z[:], 0.0)
                zlen = 128 * (MAXNR // 128) * F
                tot = plan.agg_rows * F
                for p in range(NSUB):
                    flat = aggs[s][p][:].rearrange("r f -> (r f)")
                    o = 0
                    while o < tot:
                        ln = min(zlen, tot - o)
                        # view zero tile as flat [128, ...]; dst flat slice
                        nc.sync.dma_start(
                            flat[o:o + ln].rearrange("(a b) -> a b", a=128),
                            z[:].rearrange("p s f -> p (s f)")[:, : ln // 128])
                        o += ln

            zero_agg_set(0)

            for layer in range(n_layers):
                aset = aggs[layer % 2]
                # publish g and AllGather into the shared table
                nc.sync.dma_start(
                    g_dram[:].rearrange("(p x) -> p x", p=128),
                    g[:].rearrange("p t f -> p (t f)"))
                nc.gpsimd.collective_compute(
                    "AllGather", mybir.AluOpType.bypass,
                    replica_groups=rg,
                    ins=[g_dram[:]],
                    outs=[G[0:SH * NCORES, :].rearrange("r f -> (r f)")],
                )

                # aggregation rounds
                for r in range(CAP):
                    n_r = int(plan.n_r[r])
                    gi = ipool.tile([128, MAXNR // 16], i16, tag="gi")
                    si = ipool.tile([128, MAXNR // 16], i16, tag="si")
                    nc.sync.dma_start(
                        gi[:, : n_r // 16],
                        gidx_t[:, plan.gso[(r, 0)]:plan.gso[(r, 0)] + n_r // 16])
                    nc.sync.dma_start(
                        si[:, : n_r // 16],
                        sidx_t[:, plan.sso[r]:plan.sso[r] + n_r // 16])
                    buf = mpool.tile([128, MAXNR // 128, F], f32, tag="msg")
                    o = 0
                    for c in range(NCH):
                        n_rc = int(plan.n_rc[r, c])
                        go = plan.gso[(r, c)] - plan.gso[(r, 0)]
                        nc.gpsimd.dma_gather(
                            buf[:, o // 128:(o + n_rc) // 128, :],
                            G[c * plan.row_chunk:(c + 1) * plan.row_chunk, :],
                            gi[:, go:go + n_rc // 16],
                            n_rc, n_rc, F, single_packet=False)
                        o += n_rc
                    m = n_r // NSUB
                    for j in range(NSUB):
                        nc.gpsimd.dma_scatter_add(
                            aset[j][:],
                            buf[:, j * m // 128:(j + 1) * m // 128, :],
                            si[:, j * m // 16:(j + 1) * m // 16],
                            m, m, F, queue_num=1, single_packet=False)

                # zero the other set for the next layer (overlaps rounds/tail)
                if layer < min(2, n_layers - 1):
                    zero_agg_set((layer + 1) % 2)

                # readback: sum = sum over (parity, level) of agg rows + g
                acc = spool.tile([128, T, F], f32, tag="acc")
                rbt = spool.tile([128, T, F], f32, tag="tmp")
                first = True
                for p in range(NSUB):
                    for lv in range(plan.levels):
                        dst = acc if first else rbt
                        nc.sync.dma_start(
                            dst[:],
                            aset[p][lv * SH:(lv + 1) * SH, :]
                            .rearrange("(a t) f -> a t f", a=128))
                        if not first:
                            nc.vector.tensor_tensor(
                                acc[:], acc[:], rbt[:], mybir.AluOpType.add)
                        first = False
                nc.vector.tensor_tensor(acc[:], acc[:], g[:], mybir.AluOpType.add)
                # pre = dis * acc   (node-major)
                nc.vector.tensor_tensor(acc[:], acc[:], dis_b, mybir.AluOpType.mult)

                if layer < n_layers - 1:
                    W, b = wsb[f"W{layer + 1}"], wsb[f"b{layer + 1}"]
                    g2 = spool.tile([128, T, F], f32, tag="g")
                    # chunks of 4 tiles (512 nodes) through FM
                    for t0 in range(0, T, 4):
                        nt = min(4, T - t0)
                        fm = fpool.tile([F, 4 * 128], f32, tag="fm")
                        for j in range(nt):
                            pt = ppool.tile([F, 128], f32, space="PSUM", tag="pt")
                            nc.tensor.transpose(pt[:], acc[:, t0 + j, :], ident[:])
                            nc.vector.tensor_copy(fm[:, j * 128:(j + 1) * 128], pt[:])
                        mm = ppool.tile([F, 4 * 128], f32, space="PSUM", tag="mm")
                        nc.tensor.matmul(mm[:, : nt * 128], W[:], fm[:, : nt * 128],
                                         start=True, stop=True)
                        hfm = fpool.tile([F, 4 * 128], f32, tag="hfm")
                        nc.scalar.activation(hfm[:, : nt * 128], mm[:, : nt * 128],
                                             mybir.ActivationFunctionType.Relu,
                                             bias=b[:, :1])
                        for j in range(nt):
                            pt2 = ppool.tile([128, F], f32, space="PSUM", tag="pt2")
                            nc.tensor.transpose(
                                pt2[:], hfm[:, j * 128:(j + 1) * 128], ident[:F, :F])
                            nc.vector.tensor_scalar_mul(
                                g2[:, t0 + j, :], pt2[:], dis_s[:, t0 + j:t0 + j + 1])
                    g = g2
                else:
                    W3, b3 = wsb["W3"], wsb["b3"]
                    for t0 in range(0, T, 4):
                        nt = min(4, T - t0)
                        fm = fpool.tile([F, 4 * 128], f32, tag="fm")
                        for j in range(nt):
                            pt = ppool.tile([F, 128], f32, space="PSUM", tag="pt")
                            nc.tensor.transpose(pt[:], acc[:, t0 + j, :], ident[:])
                            nc.vector.tensor_copy(fm[:, j * 128:(j + 1) * 128], pt[:])
                        mm3 = ppool.tile([1, 4 * 128], f32, space="PSUM", tag="mm")
                        nc.tensor.matmul(mm3[:, : nt * 128], W3[:], fm[:, : nt * 128],
                                         start=True, stop=True)
                        ofm = fpool.tile([1, 4 * 128], f32, tag="ofm")
                        nc.vector.tensor_scalar_add(
                            ofm[:, : nt * 128], mm3[:, : nt * 128], b3[:, :1])
                        nc.sync.dma_start(
                            out_t[t0 * 128:(t0 + nt) * 128]
                            .rearrange("(a x) -> a x", a=1),
                            ofm[:, : nt * 128])

    nc.compile()
    return nc


def kernel(**inputs):
    from concourse.bass_utils import run_bass_kernel_spmd

    x = np.asarray(inputs["x"], np.float32)
    edge_index = np.asarray(inputs["edge_index"])
    plan = Plan(x.shape[0], edge_index)
    nc = build(plan)
    in_maps = [plan.core_inputs(k, x, inputs["W1"], inputs["b1"], inputs["W2"],
                                inputs["b2"], inputs["W3"], inputs["b3"])
               for k in range(NCORES)]
    res = run_bass_kernel_spmd(nc, in_maps, core_ids=list(range(NCORES)))
    return plan.assemble(res.results)
